# revision 6
# baseline (speedup 1.0000x reference)
"""Fused AttentionMemory kernel for Trainium2, 8 NeuronCores (SPMD).

Math (reference):
    x    = LayerNorm(dec) * gamma + beta                     [B,T,D]
    q    = x @ W1^T + b1                                     [B,T,D]
    k    = enc @ W2^T + b2                                   [M,D]
    attn = LayerNorm_m(q @ k^T / sqrt(D))                    [B,T,M]
    out  = (attn + mem) * 0.5

Key reassociation: q @ k^T = x @ G with G[e,m] = sum_d W1[d,e] k[m,d],
so the per-core FLOPs drop from ~27 GF to ~11 GF.  The LayerNorm on x
folds into per-row scalars applied around a matmul on RAW dec:

    attn_pre[t,m] = rsd_t * [ (dec @ G')[t,m] - mu_t * u[m] ] + w[m]

with G' = gamma*G/sqrt(D), u = colsum(G'), w = (beta@G + b1@k^T)/sqrt(D).

Sharding: batch-parallel over B=8 for the big matmul; the G precompute is
contraction/row sharded with two small bf16 AllGathers (k^T, then G).
"""

import numpy as np

import concourse.bass as bass
import concourse.tile as tile
from concourse import mybir, bacc
from concourse.bass_utils import run_bass_kernel_spmd
from concourse.masks import make_identity

N_CORES = 8
B, T, D, M = 8, 2048, 2048, 1024
DSH = D // N_CORES          # 256 rows of k^T / rows of G per core
P = 128
ET = D // P                 # 16 contraction tiles
TT = T // P                 # 16 t tiles
DST = DSH // P              # 2 shard subtiles
NCH = M // 512              # 2 moving-dim chunks of 512
S = float(np.sqrt(D))
EPS = 1e-5
FP32 = mybir.dt.float32
BF16 = mybir.dt.bfloat16

_nc_cache = None


def build_nc():
    nc = bacc.Bacc("TRN2", target_bir_lowering=False, debug=False,
                   num_devices=N_CORES)
    dec = nc.declare_dram_parameter("dec", [T, D], FP32, isOutput=False)
    mem = nc.declare_dram_parameter("mem", [T, M], FP32, isOutput=False)
    enc = nc.declare_dram_parameter("enc", [M, D], FP32, isOutput=False)
    w2s = nc.declare_dram_parameter("w2s", [DSH, D], FP32, isOutput=False)
    w1c = nc.declare_dram_parameter("w1c", [D, DSH], FP32, isOutput=False)
    b2t = nc.declare_dram_parameter("b2t", [P, DST], FP32, isOutput=False)
    b1t = nc.declare_dram_parameter("b1t", [P, ET], FP32, isOutput=False)
    gst = nc.declare_dram_parameter("gst", [P, ET], FP32, isOutput=False)
    bet = nc.declare_dram_parameter("bet", [P, ET], FP32, isOutput=False)
    out = nc.declare_dram_parameter("out", [T, M], FP32, isOutput=True)

    with tile.TileContext(nc) as tc:
        _build(tc, nc, dec, mem, enc, w2s, w1c, b2t, b1t, gst, bet, out)
    nc.compile()
    return nc


def _build(tc, nc, dec, mem, enc, w2s, w1c, b2t, b1t, gst, bet, out):
    from contextlib import ExitStack
    ctx = ExitStack()
    rg = [list(range(N_CORES))]

    const = ctx.enter_context(tc.tile_pool(name="const", bufs=1))
    ident = const.tile([P, P], FP32)
    make_identity(nc, ident)
    identb = const.tile([P, P], BF16)
    make_identity(nc, identb)
    eps_t = const.tile([P, 1], FP32)
    nc.vector.memset(eps_t, EPS)
    eps4_t = const.tile([P, 1], FP32)
    nc.vector.memset(eps4_t, 4.0 * EPS)
    ones_b = const.tile([P, 1], BF16)
    nc.vector.memset(ones_b, 1.0)

    b2c = const.tile([P, DST], FP32)
    nc.sync.dma_start(out=b2c, in_=b2t[:, :])
    b1c = const.tile([P, ET], FP32)
    nc.sync.dma_start(out=b1c, in_=b1t[:, :])
    gsc = const.tile([P, ET], FP32)
    nc.sync.dma_start(out=gsc, in_=gst[:, :])
    bec = const.tile([P, ET], FP32)
    nc.sync.dma_start(out=bec, in_=bet[:, :])
    b1b = const.tile([P, ET], BF16)
    nc.any.tensor_copy(out=b1b, in_=b1c)
    beb = const.tile([P, ET], BF16)
    nc.any.tensor_copy(out=beb, in_=bec)

    # big-resident tiles
    big = ctx.enter_context(tc.tile_pool(name="big", bufs=1))
    gp_sb = big.tile([P, ET, M], BF16)          # G' (folded), rhs of main mm
    u_row = big.tile([1, M], BF16)              # colsum of G'
    w_bcast = big.tile([P, M], FP32)            # w[m] broadcast over partitions

    dram = ctx.enter_context(tc.tile_pool(name="dram", bufs=1, space="DRAM"))
    mm_psum = ctx.enter_context(tc.tile_pool(name="mm_psum", bufs=2, space="PSUM"))
    tp_psum = ctx.enter_context(tc.tile_pool(name="tp_psum", bufs=2, space="PSUM"))
    row_psum = ctx.enter_context(tc.tile_pool(name="row_psum", bufs=1, space="PSUM"))

    # ---------------- pre-phase: k^T shard, AllGather, G shard, AllGather
    with tc.tile_pool(name="pre", bufs=1) as pre, \
         tc.tile_pool(name="pre_in", bufs=3) as pre_in:
        # transpose full enc -> encT [e, m] bf16
        encT = pre.tile([P, ET, M], BF16)
        for j in range(M // P):                 # 8 m-tiles
            encf = pre_in.tile([P, D], FP32, tag="encf")
            nc.sync.dma_start(out=encf, in_=enc[j * P:(j + 1) * P, :])
            for k in range(ET):
                tp = tp_psum.tile([P, P], FP32, tag="tp")
                nc.tensor.transpose(tp, encf[:, k * P:(k + 1) * P], ident)
                nc.any.tensor_copy(out=encT[:, k, j * P:(j + 1) * P], in_=tp)
        # transpose W2 shard -> w2sT [e, d_local] bf16
        w2sT = pre.tile([P, ET, DSH], BF16)
        for j in range(DST):
            w2f = pre_in.tile([P, D], FP32, tag="encf")
            nc.sync.dma_start(out=w2f, in_=w2s[j * P:(j + 1) * P, :])
            for k in range(ET):
                tp = tp_psum.tile([P, P], FP32, tag="tp")
                nc.tensor.transpose(tp, w2f[:, k * P:(k + 1) * P], ident)
                nc.any.tensor_copy(out=w2sT[:, k, j * P:(j + 1) * P], in_=tp)

        # k^T shard matmul: kt[d_local, m] = sum_e W2[d,e] enc[m,e] (+b2)
        kt_in = dram.tile([DSH, M], BF16)
        for j in range(DST):
            pk = mm_psum.tile([P, M], FP32, tag="mm")
            for k in range(ET):
                for ch in range(NCH):
                    nc.tensor.matmul(
                        pk[:, ch * 512:(ch + 1) * 512],
                        lhsT=w2sT[:, k, j * P:(j + 1) * P],
                        rhs=encT[:, k, ch * 512:(ch + 1) * 512],
                        start=(k == 0), stop=(k == ET - 1))
            kt_sb = pre_in.tile([P, M], BF16, tag="ktsb")
            nc.vector.tensor_scalar(out=kt_sb, in0=pk, scalar1=b2c[:, j:j + 1],
                                    scalar2=None, op0=mybir.AluOpType.add)
            nc.sync.dma_start(out=kt_in[j * P:(j + 1) * P, :], in_=kt_sb)

        kt_all = dram.tile([D, M], BF16, addr_space="Shared")
        nc.gpsimd.collective_compute(
            "AllGather", mybir.AluOpType.bypass, replica_groups=rg,
            ins=[kt_in[:].opt()], outs=[kt_all[:].opt()])

        # G shard matmul: G[e_local, m] = sum_d W1[d, e] kt[d, m]
        ktf = pre.tile([P, ET, M], BF16)
        for k in range(ET):
            nc.sync.dma_start(out=ktf[:, k, :], in_=kt_all[k * P:(k + 1) * P, :])
        w1cf = pre.tile([P, ET, DSH], FP32)
        for k in range(ET):
            nc.sync.dma_start(out=w1cf[:, k, :], in_=w1c[k * P:(k + 1) * P, :])
        w1cb = pre.tile([P, ET, DSH], BF16)
        nc.any.tensor_copy(out=w1cb, in_=w1cf)

        g_in = dram.tile([DSH, M], BF16)
        for j in range(DST):
            pg = mm_psum.tile([P, M], FP32, tag="mm")
            for k in range(ET):
                for ch in range(NCH):
                    nc.tensor.matmul(
                        pg[:, ch * 512:(ch + 1) * 512],
                        lhsT=w1cb[:, k, j * P:(j + 1) * P],
                        rhs=ktf[:, k, ch * 512:(ch + 1) * 512],
                        start=(k == 0), stop=(k == ET - 1))
            g_sb = pre_in.tile([P, M], BF16, tag="ktsb")
            nc.any.tensor_copy(out=g_sb, in_=pg)
            nc.sync.dma_start(out=g_in[j * P:(j + 1) * P, :], in_=g_sb)

        g_all = dram.tile([D, M], BF16, addr_space="Shared")
        nc.gpsimd.collective_compute(
            "AllGather", mybir.AluOpType.bypass, replica_groups=rg,
            ins=[g_in[:].opt()], outs=[g_all[:].opt()])

        # load raw G, compute w = (beta@G + b1@kt)/S, fold G' = gamma*G/S
        graw = pre.tile([P, ET, M], BF16)
        for k in range(ET):
            nc.sync.dma_start(out=graw[:, k, :], in_=g_all[k * P:(k + 1) * P, :])

        pw = row_psum.tile([1, M], FP32, tag="row")
        for k in range(ET):
            for ch in range(NCH):
                nc.tensor.matmul(pw[:, ch * 512:(ch + 1) * 512],
                                 lhsT=beb[:, k:k + 1],
                                 rhs=graw[:, k, ch * 512:(ch + 1) * 512],
                                 start=(k == 0), stop=False)
        for k in range(ET):
            for ch in range(NCH):
                nc.tensor.matmul(pw[:, ch * 512:(ch + 1) * 512],
                                 lhsT=b1b[:, k:k + 1],
                                 rhs=ktf[:, k, ch * 512:(ch + 1) * 512],
                                 start=False, stop=(k == ET - 1))
        w_row = pre.tile([1, M], FP32)
        nc.vector.tensor_scalar_mul(out=w_row, in0=pw, scalar1=1.0 / S)
        w_dram = dram.tile([1, M], FP32)
        nc.sync.dma_start(out=w_dram, in_=w_row)
        nc.sync.dma_start(out=w_bcast,
                          in_=bass.AP(tensor=w_dram.tensor, offset=w_dram.offset,
                                      ap=[[0, P]] + list(w_dram[0, :].ap)))

        for k in range(ET):
            nc.vector.tensor_scalar_mul(out=gp_sb[:, k, :], in0=graw[:, k, :],
                                        scalar1=gsc[:, k:k + 1])

        pu = row_psum.tile([1, M], FP32, tag="row")
        for k in range(ET):
            for ch in range(NCH):
                nc.tensor.matmul(pu[:, ch * 512:(ch + 1) * 512],
                                 lhsT=ones_b,
                                 rhs=gp_sb[:, k, ch * 512:(ch + 1) * 512],
                                 start=(k == 0), stop=(k == ET - 1))
        nc.any.tensor_copy(out=u_row, in_=pu)

    # ---------------- main loop over t tiles
    with tc.tile_pool(name="mn_dec", bufs=3) as mn_dec, \
         tc.tile_pool(name="mn_io", bufs=3) as mn_io, \
         tc.tile_pool(name="mn_wk", bufs=2) as mn_wk, \
         tc.tile_pool(name="mn_st", bufs=4) as mn_st:
        for j in range(TT):
            decf = mn_dec.tile([P, D], FP32, tag="dec")
            nc.sync.dma_start(out=decf, in_=dec[j * P:(j + 1) * P, :])
            memf = mn_io.tile([P, M], FP32, tag="mem")
            nc.sync.dma_start(out=memf, in_=mem[j * P:(j + 1) * P, :])

            # LN1 stats on raw dec rows
            st = mn_st.tile([P, 4, 6], FP32, tag="st")
            dsub = decf.rearrange("p (n f) -> p n f", f=512)
            for sg in range(4):
                nc.vector.bn_stats(out=st[:, sg, :], in_=dsub[:, sg, :])
            mv = mn_st.tile([P, 2], FP32, tag="mv")
            nc.vector.bn_aggr(out=mv, in_=st)
            rsd = mn_st.tile([P, 1], FP32, tag="rsd")
            nc.scalar.activation(out=rsd, in_=mv[:, 1:2],
                                 func=mybir.ActivationFunctionType.Sqrt,
                                 bias=eps_t, scale=1.0)
            nc.vector.reciprocal(out=rsd, in_=rsd)
            negmu_b = mn_st.tile([P, 1], BF16, tag="negmu")
            nc.vector.tensor_scalar_mul(out=negmu_b, in0=mv[:, 0:1], scalar1=-1.0)
            nmp = tp_psum.tile([P, P], BF16, tag="tp")
            nc.tensor.transpose(nmp[0:1, :], negmu_b, identb)
            negmu_row = mn_st.tile([1, P], BF16, tag="negmurow")
            nc.any.tensor_copy(out=negmu_row, in_=nmp[0:1, :])

            # transpose dec tile -> decT [e, t] bf16
            decT = mn_wk.tile([P, ET, P], BF16, tag="decT")
            for k in range(ET):
                tp = tp_psum.tile([P, P], FP32, tag="tp")
                nc.tensor.transpose(tp, decf[:, k * P:(k + 1) * P], ident)
                nc.any.tensor_copy(out=decT[:, k, :], in_=tp)

            pm = mm_psum.tile([P, M], FP32, tag="mm")
            for k in range(ET):
                for ch in range(NCH):
                    nc.tensor.matmul(pm[:, ch * 512:(ch + 1) * 512],
                                     lhsT=decT[:, k, :],
                                     rhs=gp_sb[:, k, ch * 512:(ch + 1) * 512],
                                     start=(k == 0), stop=False)
            for ch in range(NCH):
                nc.tensor.matmul(pm[:, ch * 512:(ch + 1) * 512],
                                 lhsT=negmu_row,
                                 rhs=u_row[:, ch * 512:(ch + 1) * 512],
                                 start=False, stop=True)

            # epilogue: y = pm*rsd + w ; LN2 over m ; out = LN2*0.5 + mem*0.5
            y = mn_wk.tile([P, M], FP32, tag="y")
            nc.vector.tensor_scalar_mul(out=y, in0=pm, scalar1=rsd)
            nc.vector.tensor_add(out=y, in0=y, in1=w_bcast)
            st2 = mn_st.tile([P, 2, 6], FP32, tag="st2")
            ysub = y.rearrange("p (n f) -> p n f", f=512)
            for sg in range(2):
                nc.vector.bn_stats(out=st2[:, sg, :], in_=ysub[:, sg, :])
            mv2 = mn_st.tile([P, 2], FP32, tag="mv2")
            nc.vector.bn_aggr(out=mv2, in_=st2)
            rstd2h = mn_st.tile([P, 1], FP32, tag="rstd2h")
            nc.scalar.activation(out=rstd2h, in_=mv2[:, 1:2],
                                 func=mybir.ActivationFunctionType.Sqrt,
                                 bias=eps4_t, scale=4.0)
            nc.vector.reciprocal(out=rstd2h, in_=rstd2h)
            memh = mn_io.tile([P, M], FP32, tag="memh")
            nc.scalar.mul(out=memh, in_=memf, mul=0.5)
            z = mn_wk.tile([P, M], FP32, tag="z")
            nc.vector.tensor_scalar(out=z, in0=y, scalar1=mv2[:, 0:1],
                                    scalar2=rstd2h,
                                    op0=mybir.AluOpType.subtract,
                                    op1=mybir.AluOpType.mult)
            o = mn_io.tile([P, M], FP32, tag="o")
            nc.vector.tensor_add(out=o, in0=z, in1=memh)
            nc.sync.dma_start(out=out[j * P:(j + 1) * P, :], in_=o)
    ctx.close()


def _make_in_maps(dec_output, enc_out_mem_mean, mem_attn_out,
                  ln_gamma, ln_beta, W1, b1, W2, b2):
    f = np.float32
    enc = np.ascontiguousarray(enc_out_mem_mean, dtype=f)
    gst = np.ascontiguousarray((ln_gamma / S).reshape(ET, P).T, dtype=f)
    bet = np.ascontiguousarray(ln_beta.reshape(ET, P).T, dtype=f)
    b1t = np.ascontiguousarray(b1.reshape(ET, P).T, dtype=f)
    in_maps = []
    for i in range(N_CORES):
        sl = slice(i * DSH, (i + 1) * DSH)
        in_maps.append({
            "dec": np.ascontiguousarray(dec_output[i], dtype=f),
            "mem": np.ascontiguousarray(mem_attn_out[i], dtype=f),
            "enc": enc,
            "w2s": np.ascontiguousarray(W2[sl, :], dtype=f),
            "w1c": np.ascontiguousarray(W1[:, sl], dtype=f),
            "b2t": np.ascontiguousarray(b2[sl].reshape(DST, P).T, dtype=f),
            "b1t": b1t,
            "gst": gst,
            "bet": bet,
        })
    return in_maps


def kernel(**inputs) -> np.ndarray:
    global _nc_cache
    if _nc_cache is None:
        _nc_cache = build_nc()
    in_maps = _make_in_maps(**inputs)
    res = run_bass_kernel_spmd(_nc_cache, in_maps,
                               core_ids=list(range(N_CORES)))
    return np.stack([res.results[i]["out"] for i in range(N_CORES)], axis=0)
